# revision 17
# baseline (speedup 1.0000x reference)
"""BiDAF-style attention-flow kernel for Trainium2, SPMD over 8 NeuronCores.

Reference computation (per batch b):
    S[c,q] = w1.xc[c] + w2.xq[q] + (xc[c]*w3).xq[q]          (trilinear sim)
    c2q    = softmax_q(S) @ xq                                [C,E]
    q2c    = softmax_c(max_q S) @ xc                          [E]
    out    = concat([xc, c2q, xc*c2q, xc*q2c], -1)            [C,4E]

Sharding: data-parallel over batch B=32 -> 4 batches per core, no collectives.
Within a core: tile over C in 128-row tiles; matmuls run in bf16 (fp32
accumulation in PSUM), softmax statistics and outputs stay fp32.
"""

import os

# The NEFF executes on the axon-tunneled NeuronCores via PJRT; make sure jax
# can discover the axon platform even if the environment pinned cpu.
if os.environ.get("JAX_PLATFORMS") == "cpu":
    os.environ["JAX_PLATFORMS"] = ""

from contextlib import ExitStack

import numpy as np

import concourse.tile as tile
from concourse import bacc, mybir
from concourse.bass_utils import run_bass_kernel_spmd
from concourse.masks import make_identity

B, C, Q, E = 32, 2048, 128, 200
N_CORES = 8
BL = B // N_CORES          # batches per core
CT = 128                   # context rows per tile
NT = C // CT               # context tiles per batch
E1 = 128                   # contraction chunk 1
E2 = E - E1                # contraction chunk 2 (72)

F32 = mybir.dt.float32
BF16 = mybir.dt.bfloat16
Act = mybir.ActivationFunctionType
AX = mybir.AxisListType


def _build():
    nc = bacc.Bacc("TRN2", target_bir_lowering=False, debug=False,
                   enable_asserts=False)
    xc_ext = nc.declare_dram_parameter("x_contexts", [BL, C, E], F32,
                                       isOutput=False)
    xq_ext = nc.declare_dram_parameter("x_questions", [BL, Q, E], F32,
                                       isOutput=False)
    w_ext = nc.declare_dram_parameter("w_sim", [3 * E], F32, isOutput=False)
    out_ext = nc.declare_dram_parameter("out", [BL, C, 4 * E], F32,
                                        isOutput=True)

    with tile.TileContext(nc) as tc, ExitStack() as ctx:
        const = ctx.enter_context(tc.tile_pool(name="const", bufs=1))
        batchp = ctx.enter_context(tc.tile_pool(name="batch", bufs=2))
        work = ctx.enter_context(tc.tile_pool(name="work", bufs=3))
        # PSUM: 8 banks total; the four pools below use exactly 8.
        ps_t = ctx.enter_context(tc.tile_pool(name="ps_t", bufs=2, space="PSUM"))
        ps_s = ctx.enter_context(tc.tile_pool(name="ps_s", bufs=2, space="PSUM"))
        ps_pc = ctx.enter_context(tc.tile_pool(name="ps_pc", bufs=2, space="PSUM"))
        ps_misc = ctx.enter_context(tc.tile_pool(name="ps_misc", bufs=2, space="PSUM"))

        # ---- constants ----
        id_f32 = const.tile([128, 128], F32, tag="id_f32")
        make_identity(nc, id_f32[:])
        id_bf16 = const.tile([128, 128], BF16, tag="id_bf16")
        make_identity(nc, id_bf16[:])
        ones_row_bf = const.tile([1, 128], BF16, tag="ones_row_bf")
        nc.gpsimd.memset(ones_row_bf[:], 1.0)
        ones_row_f32 = const.tile([1, 128], F32, tag="ones_row_f32")
        nc.gpsimd.memset(ones_row_f32[:], 1.0)
        ones_col_f32 = const.tile([128, 1], F32, tag="ones_col_f32")
        nc.gpsimd.memset(ones_col_f32[:], 1.0)

        # w_sim per-chunk columns. Chunk 1 covers e=0..127; chunk 2 covers
        # e=72..199 (full 128 rows, overlapping chunk 1 at e=72..127) so every
        # transpose is a full [128,128] tile. The overlap rows are zeroed in
        # the chunk-2 rhs/weights so they contribute nothing to contractions.
        # col 0: w1[0:128]  col 1 rows 56:128: w1[128:200]
        # col 2: w2[0:128]  col 3 rows 56:128: w2[328:400]->w2[128:200]
        # col 4: w3[0:128]  col 5 rows 56:128: w3[128:200]
        wcols = const.tile([128, 6], F32, tag="wcols")
        nc.gpsimd.memset(wcols[:], 0.0)
        for j, lo, r0 in [(0, 0, 0), (1, 128, 56), (2, 200, 0), (3, 328, 56),
                          (4, 400, 0), (5, 528, 56)]:
            nc.gpsimd.dma_start(out=wcols[r0:128, j:j + 1],
                                in_=w_ext[lo:lo + 128 - r0])
        w2_bf = const.tile([128, 2], BF16, tag="w2_bf")
        nc.vector.tensor_copy(out=w2_bf[:], in_=wcols[:, 2:4])

        for b in range(BL):
            # ---- batch preamble: question-side tensors ----
            xq_f32 = batchp.tile([Q, E], F32, tag="xq_f32")
            nc.gpsimd.dma_start(out=xq_f32[:], in_=xq_ext[b])
            xq_bf = batchp.tile([Q, E], BF16, tag="xq_bf")
            nc.vector.tensor_copy(out=xq_bf[:], in_=xq_f32[:])

            ps_xqT = ps_t.tile([128, 256], F32, tag="ps_tr")
            nc.tensor.transpose(ps_xqT[:, 0:128], xq_f32[:, 0:E1], id_f32[:])
            nc.tensor.transpose(ps_xqT[:, 128:256], xq_f32[:, E - 128:E], id_f32[:])

            # rhs for the S matmul: w3*xqT + w1 (folds the s_c term's partner;
            # s_c itself comes from contracting xc with w1 via this bias).
            rhs1 = batchp.tile([128, Q], BF16, tag="rhs1")
            nc.scalar.activation(rhs1[:], ps_xqT[:, 0:128], Act.Identity,
                                 bias=wcols[:, 0:1], scale=wcols[:, 4:5])
            rhs2 = batchp.tile([128, Q], BF16, tag="rhs2")
            nc.scalar.activation(rhs2[:], ps_xqT[:, 128:256], Act.Identity,
                                 bias=wcols[:, 1:2], scale=wcols[:, 5:6])

            xqT1_bf = batchp.tile([128, Q], BF16, tag="xqT1_bf")
            nc.vector.tensor_copy(out=xqT1_bf[:], in_=ps_xqT[:, 0:128])
            xqT2_bf = batchp.tile([128, Q], BF16, tag="xqT2_bf")
            nc.vector.tensor_copy(out=xqT2_bf[:], in_=ps_xqT[:, 128:256])

            # s_q[q] = w2 . xq[q]  -> [1, Q] row, added into S via K=1 matmul
            ps_sq = ps_misc.tile([1, Q], F32, tag="ps_misc")
            nc.tensor.matmul(ps_sq[:], w2_bf[:, 0:1], xqT1_bf[:],
                             start=True, stop=False)
            nc.tensor.matmul(ps_sq[:], w2_bf[:, 1:2], xqT2_bf[:],
                             start=False, stop=True)
            sq_bf = batchp.tile([1, Q], BF16, tag="sq_bf")
            nc.vector.tensor_copy(out=sq_bf[:], in_=ps_sq[:])

            # ---- whole-batch output staging; block 0 doubles as the
            # resident copy of x_contexts (loaded by one big DMA). ----
            st = batchp.tile([CT, NT, 4 * E], F32, tag="st")
            Mneg = batchp.tile([CT, NT], F32, tag="Mneg")
            xc_r = xc_ext[b].rearrange("(t p) e -> p t e", p=CT)
            nc.gpsimd.dma_start(out=st[:, 0:NT // 2, 0:E],
                                in_=xc_r[:, 0:NT // 2, :])
            nc.gpsimd.dma_start(out=st[:, NT // 2:NT, 0:E],
                                in_=xc_r[:, NT // 2:NT, :])

            # q2c weights: softmax over all C of M[c]=max_q S. |M| <= ~6 so
            # exp() is safe without subtracting the global max; u_t=exp(M_t) is
            # computed per tile so the q2c accumulation runs inside phase A.
            U = batchp.tile([CT, NT], F32, tag="U")
            ps_num = ps_misc.tile([1, E], F32, tag="ps_misc")

            # ---- phase A: per-tile S, row softmax, c2q, out blocks 0..2 ----
            for t in range(NT):
                xct = st[:, t, 0:E]
                ps_xcT = ps_t.tile([128, 256], F32, tag="ps_tr")
                nc.tensor.transpose(ps_xcT[:, 0:128], st[:, t, 0:E1], id_f32[:])
                nc.tensor.transpose(ps_xcT[:, 128:256], st[:, t, E - 128:E],
                                    id_f32[:])
                xcT = work.tile([128, 2 * CT], BF16, tag="xcT")
                nc.vector.tensor_copy(out=xcT[:], in_=ps_xcT[:])

                ps_S = ps_s.tile([CT, Q], F32, tag="ps_S")
                nc.tensor.matmul(ps_S[:], xcT[:, 0:CT], rhs1[:],
                                 start=True, stop=False)
                nc.tensor.matmul(ps_S[:], xcT[:, CT:2 * CT], rhs2[:],
                                 start=False, stop=False)
                nc.tensor.matmul(ps_S[:], ones_row_bf[:], sq_bf[:],
                                 start=False, stop=True)

                negm = Mneg[:, t:t + 1]
                nc.vector.reduce_max(out=negm, in_=ps_S[:], axis=AX.X,
                                     negate=True)
                nc.scalar.activation(U[:, t:t + 1], negm, Act.Exp,
                                     bias=0.0, scale=-1.0)
                nc.tensor.matmul(ps_num[:], U[:, t:t + 1], st[:, t, 0:E],
                                 start=(t == 0), stop=(t == NT - 1))

                P_bf = work.tile([CT, Q], BF16, tag="P_bf")
                Z = work.tile([CT, 1], F32, tag="Z")
                nc.scalar.activation(P_bf[:], ps_S[:], Act.Exp,
                                     bias=negm, scale=1.0, accum_out=Z[:])

                ps_PT = ps_pc.tile([Q, CT], BF16, tag="ps_pc")
                nc.tensor.transpose(ps_PT[:], P_bf[:], id_bf16[:])
                PT_bf = work.tile([Q, CT], BF16, tag="PT_bf")
                nc.vector.tensor_copy(out=PT_bf[:], in_=ps_PT[:])

                ps_c2q = ps_pc.tile([CT, E], F32, tag="ps_pc")
                nc.tensor.matmul(ps_c2q[:], PT_bf[:], xq_bf[:],
                                 start=True, stop=True)

                rz = work.tile([CT, 1], F32, tag="rz")
                nc.vector.reciprocal(rz[:], Z[:])

                nc.scalar.activation(st[:, t, E:2 * E], ps_c2q[:], Act.Copy,
                                     bias=0.0, scale=rz[:])
                nc.gpsimd.tensor_mul(st[:, t, 2 * E:3 * E], st[:, t, E:2 * E],
                                     xct)

            # ---- phase B: q2c normalization + out block 3 ----
            ps_den = ps_misc.tile([1, NT], F32, tag="ps_misc")
            nc.tensor.matmul(ps_den[:], ones_col_f32[:], U[:],
                             start=True, stop=True)
            den = work.tile([1, 1], F32, tag="den")
            nc.vector.reduce_sum(out=den[:], in_=ps_den[:], axis=AX.X)
            rd = work.tile([1, 1], F32, tag="rd")
            nc.vector.reciprocal(rd[:], den[:])
            q2c_row = batchp.tile([1, E], F32, tag="q2c_row")
            nc.scalar.activation(q2c_row[:], ps_num[:], Act.Copy,
                                 bias=0.0, scale=rd[:])
            ps_bc = ps_misc.tile([128, E], F32, tag="ps_misc")
            nc.tensor.matmul(ps_bc[:], ones_row_f32[:], q2c_row[:],
                             start=True, stop=True)
            q2c_bc = batchp.tile([128, E], F32, tag="q2c_bc")
            nc.vector.tensor_copy(out=q2c_bc[:], in_=ps_bc[:])

            H = NT // 2
            out_r = out_ext[b].rearrange("(t p) e -> p t e", p=CT)
            for t in range(NT):
                nc.gpsimd.tensor_mul(st[:, t, 3 * E:4 * E], st[:, t, 0:E],
                                     q2c_bc[:])
                if t == H - 1:
                    nc.sync.dma_start(out=out_r[:, 0:H, :], in_=st[:, 0:H, :])
            nc.sync.dma_start(out=out_r[:, H:NT, :], in_=st[:, H:NT, :])

    nc.compile()
    return nc


_CACHE = {}


def _get_nc():
    if "nc" not in _CACHE:
        _CACHE["nc"] = _build()
    return _CACHE["nc"]


def _in_maps(x_contexts, x_questions, w_sim):
    x_contexts = np.ascontiguousarray(x_contexts, dtype=np.float32)
    x_questions = np.ascontiguousarray(x_questions, dtype=np.float32)
    w_sim = np.ascontiguousarray(w_sim, dtype=np.float32)
    maps = []
    for i in range(N_CORES):
        sl = slice(i * BL, (i + 1) * BL)
        maps.append({
            "x_contexts": x_contexts[sl],
            "x_questions": x_questions[sl],
            "w_sim": w_sim,
        })
    return maps


def _runner():
    """Build (once) a jitted SPMD executor over the 8 axon NeuronCores.

    Mirrors bass2jax.run_bass_via_pjrt's multi-core path, but caches the
    jitted callable so repeated kernel() calls and benchmarking reuse the
    compiled NEFF instead of recompiling per call.
    """
    if "runner" in _CACHE:
        return _CACHE["runner"]
    import jax
    from jax.sharding import Mesh, PartitionSpec
    from jax.experimental.shard_map import shard_map
    from concourse import bass2jax

    nc = _get_nc()
    bass2jax.install_neuronx_cc_hook()

    partition_name = (nc.partition_id_tensor.name
                      if nc.partition_id_tensor else None)
    in_names, out_names, out_avals = [], [], []
    for alloc in nc.m.functions[0].allocations:
        if not isinstance(alloc, mybir.MemoryLocationSet):
            continue
        name = alloc.memorylocations[0].name
        if alloc.kind == "ExternalInput":
            if name != partition_name:
                in_names.append(name)
        elif alloc.kind == "ExternalOutput":
            out_names.append(name)
            out_avals.append(jax.core.ShapedArray(
                tuple(alloc.tensor_shape), mybir.dt.np(alloc.dtype)))
    n_params = len(in_names)
    all_in_names = in_names + out_names
    if partition_name is not None:
        all_in_names = all_in_names + [partition_name]
    all_in_names = tuple(all_in_names)

    def _body(*args):
        operands = list(args)
        if partition_name is not None:
            operands.append(bass2jax.partition_id_tensor())
        return tuple(bass2jax._bass_exec_p.bind(
            *operands,
            out_avals=tuple(out_avals),
            in_names=all_in_names,
            out_names=tuple(out_names),
            lowering_input_output_aliases=(),
            sim_require_finite=True,
            sim_require_nnan=True,
            nc=nc,
        ))

    devices = jax.devices()[:N_CORES]
    assert len(devices) == N_CORES, devices
    mesh = Mesh(np.asarray(devices), ("core",))
    n_outs = len(out_names)
    fn = jax.jit(
        shard_map(_body, mesh=mesh,
                  in_specs=(PartitionSpec("core"),) * (n_params + n_outs),
                  out_specs=(PartitionSpec("core"),) * n_outs,
                  check_rep=False),
        donate_argnums=tuple(range(n_params, n_params + n_outs)),
        keep_unused=True,
    )
    _CACHE["runner"] = (fn, mesh, in_names, out_names, out_avals)
    return _CACHE["runner"]


def _concat_inputs(x_contexts, x_questions, w_sim):
    fn, mesh, in_names, out_names, out_avals = _runner()
    maps = _in_maps(x_contexts, x_questions, w_sim)
    return [np.concatenate([m[n] for m in maps], axis=0) for n in in_names]


def _zero_outs():
    _, _, _, _, out_avals = _runner()
    return [np.zeros((N_CORES * a.shape[0], *a.shape[1:]), a.dtype)
            for a in out_avals]


def _run(x_contexts, x_questions, w_sim, trace=False):
    """Execute once; returns (full_output, exec results namespace)."""
    fn, mesh, in_names, out_names, out_avals = _runner()
    outs = fn(*_concat_inputs(x_contexts, x_questions, w_sim), *_zero_outs())
    out = np.asarray(outs[out_names.index("out")])
    return out, outs


def _bench(x_contexts, x_questions, w_sim, iters=32):
    """Pipelined on-device timing: inputs stay resident on the devices, each
    iteration's donated output buffer is the previous iteration's result.
    Returns (avg_seconds_per_iter, full_output_of_last_iter)."""
    import time as _time
    import jax
    from jax.sharding import NamedSharding, PartitionSpec

    fn, mesh, in_names, out_names, out_avals = _runner()
    sh = NamedSharding(mesh, PartitionSpec("core"))
    d_ins = [jax.device_put(a, sh)
             for a in _concat_inputs(x_contexts, x_questions, w_sim)]
    outs = fn(*d_ins, *_zero_outs())          # warm-up / compile
    jax.block_until_ready(outs)
    t0 = _time.perf_counter()
    for _ in range(iters):
        outs = fn(*d_ins, *outs)
    jax.block_until_ready(outs)
    t1 = _time.perf_counter()
    out = np.asarray(outs[out_names.index("out")])
    return (t1 - t0) / iters, out


def kernel(x_contexts, x_questions, w_sim):
    out, _ = _run(x_contexts, x_questions, w_sim)
    return out


# revision 21
# speedup vs baseline: 91.7478x; 91.7478x over previous
"""BiDAF-style attention-flow kernel for Trainium2, SPMD over 8 NeuronCores.

Reference computation (per batch b):
    S[c,q] = w1.xc[c] + w2.xq[q] + (xc[c]*w3).xq[q]          (trilinear sim)
    c2q    = softmax_q(S) @ xq                                [C,E]
    q2c    = softmax_c(max_q S) @ xc                          [E]
    out    = concat([xc, c2q, xc*c2q, xc*q2c], -1)            [C,4E]

Sharding: data-parallel over batch B=32 -> 4 batches per core, no collectives.
Within a core: tile over C in 128-row tiles; matmuls run in bf16 (fp32
accumulation in PSUM), softmax statistics and outputs stay fp32.
"""

import os

# The NEFF executes on the axon-tunneled NeuronCores via PJRT; make sure jax
# can discover the axon platform even if the environment pinned cpu.
if os.environ.get("JAX_PLATFORMS") == "cpu":
    os.environ["JAX_PLATFORMS"] = ""

from contextlib import ExitStack

import numpy as np

import concourse.tile as tile
from concourse import bacc, mybir
from concourse.bass_utils import run_bass_kernel_spmd
from concourse.masks import make_identity

B, C, Q, E = 32, 2048, 128, 200
N_CORES = 8
BL = B // N_CORES          # batches per core
CT = 128                   # context rows per tile
NT = C // CT               # context tiles per batch
E1 = 128                   # contraction chunk 1
E2 = E - E1                # contraction chunk 2 (72)

F32 = mybir.dt.float32
BF16 = mybir.dt.bfloat16
Act = mybir.ActivationFunctionType
AX = mybir.AxisListType


def _build():
    nc = bacc.Bacc("TRN2", target_bir_lowering=False, debug=False,
                   enable_asserts=False)
    xc_ext = nc.declare_dram_parameter("x_contexts", [BL, C, E], F32,
                                       isOutput=False)
    xq_ext = nc.declare_dram_parameter("x_questions", [BL, Q, E], F32,
                                       isOutput=False)
    w_ext = nc.declare_dram_parameter("w_sim", [3 * E], F32, isOutput=False)
    out_ext = nc.declare_dram_parameter("out", [BL, C, 4 * E], F32,
                                        isOutput=True)

    with tile.TileContext(nc) as tc, ExitStack() as ctx:
        const = ctx.enter_context(tc.tile_pool(name="const", bufs=1))
        batchp = ctx.enter_context(tc.tile_pool(name="batch", bufs=2))
        stp = ctx.enter_context(tc.tile_pool(name="stp", bufs=3))
        work = ctx.enter_context(tc.tile_pool(name="work", bufs=3))
        # PSUM: 8 banks total; the four pools below use exactly 8.
        ps_t = ctx.enter_context(tc.tile_pool(name="ps_t", bufs=2, space="PSUM"))
        ps_s = ctx.enter_context(tc.tile_pool(name="ps_s", bufs=2, space="PSUM"))
        ps_pc = ctx.enter_context(tc.tile_pool(name="ps_pc", bufs=2, space="PSUM"))
        ps_misc = ctx.enter_context(tc.tile_pool(name="ps_misc", bufs=2, space="PSUM"))

        # ---- constants ----
        id_f32 = const.tile([128, 128], F32, tag="id_f32")
        make_identity(nc, id_f32[:])
        id_bf16 = const.tile([128, 128], BF16, tag="id_bf16")
        make_identity(nc, id_bf16[:])
        ones_row_bf = const.tile([1, 128], BF16, tag="ones_row_bf")
        nc.gpsimd.memset(ones_row_bf[:], 1.0)
        ones_row_f32 = const.tile([1, 128], F32, tag="ones_row_f32")
        nc.gpsimd.memset(ones_row_f32[:], 1.0)
        ones_col_bf = const.tile([128, 1], BF16, tag="ones_col_bf")
        nc.gpsimd.memset(ones_col_bf[:], 1.0)

        # w_sim per-chunk columns. Chunk 1 covers e=0..127; chunk 2 covers
        # e=72..199 (full 128 rows, overlapping chunk 1 at e=72..127) so every
        # transpose is a full [128,128] tile. The overlap rows are zeroed in
        # the chunk-2 rhs/weights so they contribute nothing to contractions.
        # col 0: w1[0:128]  col 1 rows 56:128: w1[128:200]
        # col 2: w2[0:128]  col 3 rows 56:128: w2[328:400]->w2[128:200]
        # col 4: w3[0:128]  col 5 rows 56:128: w3[128:200]
        wcols = const.tile([128, 6], F32, tag="wcols")
        nc.gpsimd.memset(wcols[:], 0.0)
        for j, lo, r0 in [(0, 0, 0), (1, 128, 56), (2, 200, 0), (3, 328, 56),
                          (4, 400, 0), (5, 528, 56)]:
            nc.sync.dma_start(out=wcols[r0:128, j:j + 1],
                                in_=w_ext[lo:lo + 128 - r0])
        act_warm = const.tile([1, 1], F32, tag="act_warm")
        nc.scalar.activation(act_warm[:], ones_row_f32[0:1, 0:1], Act.Exp)
        w2_bf = const.tile([128, 2], BF16, tag="w2_bf")
        nc.vector.tensor_copy(out=w2_bf[:], in_=wcols[:, 2:4])

        for b in range(BL):
            # ---- batch preamble: question-side tensors ----
            xq_f32 = batchp.tile([Q, E], F32, tag="xq_f32")
            nc.sync.dma_start(out=xq_f32[:], in_=xq_ext[b])
            xq_bf = batchp.tile([Q, E], BF16, tag="xq_bf")
            nc.vector.tensor_copy(out=xq_bf[:], in_=xq_f32[:])

            ps_xqT = ps_t.tile([128, 256], F32, tag="ps_tr")
            nc.tensor.transpose(ps_xqT[:, 0:128], xq_f32[:, 0:E1], id_f32[:])
            nc.tensor.transpose(ps_xqT[:, 128:256], xq_f32[:, E - 128:E], id_f32[:])

            # rhs for the S matmul: w3*xqT + w1 (folds the s_c term's partner;
            # s_c itself comes from contracting xc with w1 via this bias).
            rhs1 = batchp.tile([128, Q], BF16, tag="rhs1")
            nc.scalar.activation(rhs1[:], ps_xqT[:, 0:128], Act.Identity,
                                 bias=wcols[:, 0:1], scale=wcols[:, 4:5])
            rhs2 = batchp.tile([128, Q], BF16, tag="rhs2")
            nc.scalar.activation(rhs2[:], ps_xqT[:, 128:256], Act.Identity,
                                 bias=wcols[:, 1:2], scale=wcols[:, 5:6])

            xqT1_bf = batchp.tile([128, Q], BF16, tag="xqT1_bf")
            nc.vector.tensor_copy(out=xqT1_bf[:], in_=ps_xqT[:, 0:128])
            xqT2_bf = batchp.tile([128, Q], BF16, tag="xqT2_bf")
            nc.vector.tensor_copy(out=xqT2_bf[:], in_=ps_xqT[:, 128:256])

            # s_q[q] = w2 . xq[q]  -> [1, Q] row, added into S via K=1 matmul
            ps_sq = ps_misc.tile([1, Q], F32, tag="ps_misc")
            nc.tensor.matmul(ps_sq[:], w2_bf[:, 0:1], xqT1_bf[:],
                             start=True, stop=False)
            nc.tensor.matmul(ps_sq[:], w2_bf[:, 1:2], xqT2_bf[:],
                             start=False, stop=True)
            sq_bf = batchp.tile([1, Q], BF16, tag="sq_bf")
            nc.vector.tensor_copy(out=sq_bf[:], in_=ps_sq[:])

            # ---- whole-batch output staging; block 0 doubles as the
            # resident copy of x_contexts (loaded by one big DMA). ----
            st = stp.tile([CT, NT, 4 * E], F32, tag="st")
            Mneg = batchp.tile([CT, NT], F32, tag="Mneg")
            xc_r = xc_ext[b].rearrange("(t p) e -> p t e", p=CT)
            out_r = out_ext[b].rearrange("(t p) e -> p t e", p=CT)
            for q in range(0, NT, 4):
                nc.gpsimd.dma_start(out=st[:, q:q + 4, 0:E],
                                    in_=xc_r[:, q:q + 4, :])

            # q2c weights: softmax over all C of M[c]=max_q S. |M| <= ~6 so
            # exp() is safe without subtracting the global max; u_t=exp(M_t) is
            # computed per tile so the q2c accumulation runs inside phase A.
            U = batchp.tile([CT, NT], BF16, tag="U")
            ps_num = ps_misc.tile([1, E], F32, tag="ps_misc")

            # ---- phase A: per-tile S, row softmax, c2q, out blocks 0..2 ----
            for t in range(NT):
                xct = st[:, t, 0:E]
                ps_xcT = ps_t.tile([128, 256], F32, tag="ps_tr")
                nc.tensor.transpose(ps_xcT[:, 0:128], st[:, t, 0:E1], id_f32[:])
                nc.tensor.transpose(ps_xcT[:, 128:256], st[:, t, E - 128:E],
                                    id_f32[:])
                xcT = work.tile([128, 2 * CT], BF16, tag="xcT")
                nc.vector.tensor_copy(out=xcT[:], in_=ps_xcT[:])

                ps_S = ps_s.tile([CT, Q], F32, tag="ps_S")
                nc.tensor.matmul(ps_S[:], xcT[:, 0:CT], rhs1[:],
                                 start=True, stop=False)
                nc.tensor.matmul(ps_S[:], xcT[:, CT:2 * CT], rhs2[:],
                                 start=False, stop=False)
                nc.tensor.matmul(ps_S[:], ones_row_bf[:], sq_bf[:],
                                 start=False, stop=True)

                negm = Mneg[:, t:t + 1]
                nc.vector.reduce_max(out=negm, in_=ps_S[:], axis=AX.X,
                                     negate=True)
                nc.scalar.activation(U[:, t:t + 1], negm, Act.Exp,
                                     bias=0.0, scale=-1.0)
                xc_bf = work.tile([CT, E], BF16, tag="xc_bf")
                nc.gpsimd.tensor_copy(out=xc_bf[:], in_=xct)
                nc.tensor.matmul(ps_num[:], U[:, t:t + 1], xc_bf[:],
                                 start=(t == 0), stop=(t == NT - 1))

                P_bf = work.tile([CT, Q], BF16, tag="P_bf")
                Z = work.tile([CT, 1], F32, tag="Z")
                nc.scalar.activation(P_bf[:], ps_S[:], Act.Exp,
                                     bias=negm, scale=1.0, accum_out=Z[:])

                ps_PT = ps_pc.tile([Q, CT], BF16, tag="ps_pc")
                nc.tensor.transpose(ps_PT[:], P_bf[:], id_bf16[:])
                PT_bf = work.tile([Q, CT], BF16, tag="PT_bf")
                nc.vector.tensor_copy(out=PT_bf[:], in_=ps_PT[:])

                ps_c2q = ps_pc.tile([CT, E], F32, tag="ps_pc")
                nc.tensor.matmul(ps_c2q[:], PT_bf[:], xq_bf[:],
                                 start=True, stop=True)

                rz = work.tile([CT, 1], F32, tag="rz")
                nc.vector.reciprocal(rz[:], Z[:])

                nc.scalar.activation(st[:, t, E:2 * E], ps_c2q[:], Act.Copy,
                                     bias=0.0, scale=rz[:])
                nc.gpsimd.tensor_mul(st[:, t, 2 * E:3 * E], st[:, t, E:2 * E],
                                     xct)
                if t % 4 == 3:
                    nc.sync.dma_start(
                        out=out_r[:, t - 3:t + 1, 0:3 * E],
                        in_=st[:, t - 3:t + 1, 0:3 * E])

            # ---- phase B: q2c normalization + out block 3 ----
            ps_den = ps_misc.tile([1, NT], F32, tag="ps_misc")
            nc.tensor.matmul(ps_den[:], ones_col_bf[:], U[:],
                             start=True, stop=True)
            den = work.tile([1, 1], F32, tag="den")
            nc.vector.reduce_sum(out=den[:], in_=ps_den[:], axis=AX.X)
            rd = work.tile([1, 1], F32, tag="rd")
            nc.vector.reciprocal(rd[:], den[:])
            q2c_row = batchp.tile([1, E], F32, tag="q2c_row")
            nc.scalar.activation(q2c_row[:], ps_num[:], Act.Copy,
                                 bias=0.0, scale=rd[:])
            ps_bc = ps_misc.tile([128, E], F32, tag="ps_misc")
            nc.tensor.matmul(ps_bc[:], ones_row_f32[:], q2c_row[:],
                             start=True, stop=True)
            q2c_bc = batchp.tile([128, E], F32, tag="q2c_bc")
            nc.vector.tensor_copy(out=q2c_bc[:], in_=ps_bc[:])

            for t in range(NT):
                nc.gpsimd.tensor_mul(st[:, t, 3 * E:4 * E], st[:, t, 0:E],
                                     q2c_bc[:])
                if t % 4 == 3:
                    nc.sync.dma_start(out=out_r[:, t - 3:t + 1, 3 * E:4 * E],
                                      in_=st[:, t - 3:t + 1, 3 * E:4 * E])

    nc.compile()
    return nc


_CACHE = {}


def _get_nc():
    if "nc" not in _CACHE:
        _CACHE["nc"] = _build()
    return _CACHE["nc"]


def _in_maps(x_contexts, x_questions, w_sim):
    x_contexts = np.ascontiguousarray(x_contexts, dtype=np.float32)
    x_questions = np.ascontiguousarray(x_questions, dtype=np.float32)
    w_sim = np.ascontiguousarray(w_sim, dtype=np.float32)
    maps = []
    for i in range(N_CORES):
        sl = slice(i * BL, (i + 1) * BL)
        maps.append({
            "x_contexts": x_contexts[sl],
            "x_questions": x_questions[sl],
            "w_sim": w_sim,
        })
    return maps


def _runner():
    """Build (once) a jitted SPMD executor over the 8 axon NeuronCores.

    Mirrors bass2jax.run_bass_via_pjrt's multi-core path, but caches the
    jitted callable so repeated kernel() calls and benchmarking reuse the
    compiled NEFF instead of recompiling per call.
    """
    if "runner" in _CACHE:
        return _CACHE["runner"]
    import jax
    from jax.sharding import Mesh, PartitionSpec
    from jax.experimental.shard_map import shard_map
    from concourse import bass2jax

    nc = _get_nc()
    bass2jax.install_neuronx_cc_hook()

    partition_name = (nc.partition_id_tensor.name
                      if nc.partition_id_tensor else None)
    in_names, out_names, out_avals = [], [], []
    for alloc in nc.m.functions[0].allocations:
        if not isinstance(alloc, mybir.MemoryLocationSet):
            continue
        name = alloc.memorylocations[0].name
        if alloc.kind == "ExternalInput":
            if name != partition_name:
                in_names.append(name)
        elif alloc.kind == "ExternalOutput":
            out_names.append(name)
            out_avals.append(jax.core.ShapedArray(
                tuple(alloc.tensor_shape), mybir.dt.np(alloc.dtype)))
    n_params = len(in_names)
    all_in_names = in_names + out_names
    if partition_name is not None:
        all_in_names = all_in_names + [partition_name]
    all_in_names = tuple(all_in_names)

    def _body(*args):
        operands = list(args)
        if partition_name is not None:
            operands.append(bass2jax.partition_id_tensor())
        return tuple(bass2jax._bass_exec_p.bind(
            *operands,
            out_avals=tuple(out_avals),
            in_names=all_in_names,
            out_names=tuple(out_names),
            lowering_input_output_aliases=(),
            sim_require_finite=True,
            sim_require_nnan=True,
            nc=nc,
        ))

    devices = jax.devices()[:N_CORES]
    assert len(devices) == N_CORES, devices
    mesh = Mesh(np.asarray(devices), ("core",))
    n_outs = len(out_names)
    fn = jax.jit(
        shard_map(_body, mesh=mesh,
                  in_specs=(PartitionSpec("core"),) * (n_params + n_outs),
                  out_specs=(PartitionSpec("core"),) * n_outs,
                  check_rep=False),
        donate_argnums=tuple(range(n_params, n_params + n_outs)),
        keep_unused=True,
    )
    _CACHE["runner"] = (fn, mesh, in_names, out_names, out_avals)
    return _CACHE["runner"]


def _concat_inputs(x_contexts, x_questions, w_sim):
    fn, mesh, in_names, out_names, out_avals = _runner()
    maps = _in_maps(x_contexts, x_questions, w_sim)
    return [np.concatenate([m[n] for m in maps], axis=0) for n in in_names]


def _zero_outs():
    _, _, _, _, out_avals = _runner()
    return [np.zeros((N_CORES * a.shape[0], *a.shape[1:]), a.dtype)
            for a in out_avals]


def _run(x_contexts, x_questions, w_sim, trace=False):
    """Execute once; returns (full_output, exec results namespace)."""
    fn, mesh, in_names, out_names, out_avals = _runner()
    outs = fn(*_concat_inputs(x_contexts, x_questions, w_sim), *_zero_outs())
    out = np.asarray(outs[out_names.index("out")])
    return out, outs


def _bench_chain(x_contexts, x_questions, w_sim, chain=8, reps=4):
    """Chain `chain` NEFF executions inside ONE jitted call (output buffers
    feed the next execution's donated out operands), so per-dispatch axon
    overhead is paid once per `chain` device executions. Returns
    (marginal_seconds_per_exec, chain_call_seconds)."""
    import time as _time
    import jax
    from jax.sharding import Mesh, PartitionSpec, NamedSharding
    from jax.experimental.shard_map import shard_map
    from concourse import bass2jax

    nc = _get_nc()
    fn1, mesh, in_names, out_names, out_avals = _runner()
    partition_name = (nc.partition_id_tensor.name
                      if nc.partition_id_tensor else None)
    n_params = len(in_names)
    all_in_names = in_names + out_names
    if partition_name is not None:
        all_in_names = all_in_names + [partition_name]
    all_in_names = tuple(all_in_names)

    def _make_chained(ch):
        def _bodyN(*args):
            ins = list(args[:n_params])
            outs = list(args[n_params:])
            for _ in range(ch):
                operands = ins + outs
                if partition_name is not None:
                    operands.append(bass2jax.partition_id_tensor())
                outs = list(bass2jax._bass_exec_p.bind(
                    *operands,
                    out_avals=tuple(out_avals),
                    in_names=all_in_names,
                    out_names=tuple(out_names),
                    lowering_input_output_aliases=(),
                    sim_require_finite=True,
                    sim_require_nnan=True,
                    nc=nc,
                ))
            return tuple(outs)
        n_outs = len(out_names)
        return jax.jit(
            shard_map(_bodyN, mesh=mesh,
                      in_specs=(PartitionSpec("core"),) * (n_params + n_outs),
                      out_specs=(PartitionSpec("core"),) * n_outs,
                      check_rep=False),
            donate_argnums=tuple(range(n_params, n_params + n_outs)),
            keep_unused=True)

    sh = NamedSharding(mesh, PartitionSpec("core"))
    d_ins = [jax.device_put(a, sh)
             for a in _concat_inputs(x_contexts, x_questions, w_sim)]

    def timed(fn, reps):
        outs = fn(*d_ins, *_zero_outs())   # compile + warm
        jax.block_until_ready(outs)
        ts = []
        for _ in range(reps):
            t0 = _time.perf_counter()
            outs = fn(*d_ins, *outs)
            jax.block_until_ready(outs)
            ts.append(_time.perf_counter() - t0)
        return min(ts)

    t1 = timed(_make_chained(1), reps)
    tN = timed(_make_chained(chain), reps)
    marginal = (tN - t1) / (chain - 1)
    return marginal, t1, tN


def _bench(x_contexts, x_questions, w_sim, iters=32):
    """Pipelined on-device timing: inputs stay resident on the devices, each
    iteration's donated output buffer is the previous iteration's result.
    Returns (avg_seconds_per_iter, full_output_of_last_iter)."""
    import time as _time
    import jax
    from jax.sharding import NamedSharding, PartitionSpec

    fn, mesh, in_names, out_names, out_avals = _runner()
    sh = NamedSharding(mesh, PartitionSpec("core"))
    d_ins = [jax.device_put(a, sh)
             for a in _concat_inputs(x_contexts, x_questions, w_sim)]
    outs = fn(*d_ins, *_zero_outs())          # warm-up / compile
    jax.block_until_ready(outs)
    t0 = _time.perf_counter()
    for _ in range(iters):
        outs = fn(*d_ins, *outs)
    jax.block_until_ready(outs)
    t1 = _time.perf_counter()
    out = np.asarray(outs[out_names.index("out")])
    return (t1 - t0) / iters, out


def kernel(x_contexts, x_questions, w_sim):
    out, _ = _run(x_contexts, x_questions, w_sim)
    return out
